# revision 1
# baseline (speedup 1.0000x reference)
"""CustomSAGEConv on 8 Trainium2 NeuronCores.

out = x @ W_self.T + b_self + segment_mean(msgs[row], col)
where msgs = x @ W_msg.T + b_msg.

Since the message projection is linear, it commutes with the mean:
  agg = segment_sum(x[row] * rv[col], col) @ W_msg.T + (deg>0)*b_msg
with rv = 1/max(deg,1), so the device only needs a weighted raw-feature
segment sum, then one [128,64]-weight projection per 128-node block:
  out_block = [agg_feats; x_feats]^T . [W_msg.T; W_self.T]

Sharding: destination nodes are padded to 50176 = 8 cores x 49 groups x 128
nodes. Edges are bucketed by destination group on the host (this is the edge
partitioning step), so each core exclusively owns its 49 output groups and no
cross-core reduction is needed. Per group, the weighted segment sum is
computed as a one-hot matmul accumulated in PSUM:
  psum[64f, 128d] += xg[128e, 64f].T @ (onehot(dest) * rv)[128e, 128d]
with source rows brought in by dma_gather (4 SWDGE queues round-robin).
dma_gather indices are int16, so x is split at row 32768 into lo/hi tables
and each group's edges are partitioned into a lo list and a hi list (two
gathers per group).
"""

import sys

for _p in ("/opt/trn_rl_repo", "/root/.axon_site/_ro/trn_rl_repo"):
    if _p not in sys.path:
        sys.path.insert(0, _p)

import numpy as np

P = 128
D = 64
NC = 8
SPLIT = 32768

_CACHE = {}


def _ceil_div(a, b):
    return (a + b - 1) // b


def _build_bass(T_A, T_B, GPC, n_lo, n_hi, with_bias, repeat=1):
    import concourse.mybir as mybir
    import concourse.tile as tile
    from concourse import bacc

    T = T_A + T_B
    NPC = GPC * P  # nodes per core

    nc = bacc.Bacc(num_swdge_queues=4)
    f32 = mybir.dt.float32
    x_lo = nc.declare_dram_parameter("x_lo", [n_lo, D], f32, isOutput=False)
    x_hi = nc.declare_dram_parameter("x_hi", [n_hi, D], f32, isOutput=False)
    xT = nc.declare_dram_parameter("xT", [D, NPC], f32, isOutput=False)
    idxA = nc.declare_dram_parameter("idxA", [P, GPC * T_A * 8], mybir.dt.int16, isOutput=False)
    idxB = nc.declare_dram_parameter("idxB", [P, GPC * T_B * 8], mybir.dt.int16, isOutput=False)
    dest = nc.declare_dram_parameter("dest", [P, GPC * T], f32, isOutput=False)
    rve = nc.declare_dram_parameter("rve", [P, GPC * T], f32, isOutput=False)
    Wcat = nc.declare_dram_parameter("Wcat", [2 * D, D], f32, isOutput=False)
    if with_bias:
        bias = nc.declare_dram_parameter("bias", [NPC, D], f32, isOutput=False)
    out = nc.declare_dram_parameter("out", [NPC, D], f32, isOutput=True)

    with tile.TileContext(nc) as tc:
        with (
            tc.tile_pool(name="const", bufs=1) as cpool,
            tc.tile_pool(name="gather", bufs=3) as gpool,
            tc.tile_pool(name="oh", bufs=8) as ohpool,
            tc.tile_pool(name="small", bufs=3) as spool,
            tc.tile_pool(name="psum1", bufs=2, space="PSUM") as p1pool,
            tc.tile_pool(name="psum2", bufs=3, space="PSUM") as p2pool,
        ):
            iota_i = cpool.tile([P, P], mybir.dt.int32)
            nc.gpsimd.iota(iota_i[:], pattern=[[1, P]], base=0, channel_multiplier=0)
            iota_f = cpool.tile([P, P], f32)
            nc.vector.tensor_copy(iota_f[:], iota_i[:])

            idxA_sb = cpool.tile([P, GPC * T_A * 8], mybir.dt.int16)
            nc.sync.dma_start(out=idxA_sb[:], in_=idxA[:])
            idxB_sb = cpool.tile([P, GPC * T_B * 8], mybir.dt.int16)
            nc.sync.dma_start(out=idxB_sb[:], in_=idxB[:])
            dest_sb = cpool.tile([P, GPC * T], f32)
            nc.sync.dma_start(out=dest_sb[:], in_=dest[:])
            rve_sb = cpool.tile([P, GPC * T], f32)
            nc.sync.dma_start(out=rve_sb[:], in_=rve[:])
            Wcat_sb = cpool.tile([2 * D, D], f32)
            nc.sync.dma_start(out=Wcat_sb[:], in_=Wcat[:])

            for it in range(GPC * repeat):
                g = it % GPC
                xga = gpool.tile([P, T_A, D], f32, tag="xga")
                nc.gpsimd.dma_gather(
                    out_ap=xga[:],
                    in_ap=x_lo[:],
                    idxs_ap=idxA_sb[:, g * T_A * 8:(g + 1) * T_A * 8],
                    num_idxs=T_A * P,
                    num_idxs_reg=T_A * P,
                    elem_size=D,
                    single_packet=False,
                    queue_num=g % 4,
                )
                xgb = gpool.tile([P, T_B, D], f32, tag="xgb")
                nc.gpsimd.dma_gather(
                    out_ap=xgb[:],
                    in_ap=x_hi[:],
                    idxs_ap=idxB_sb[:, g * T_B * 8:(g + 1) * T_B * 8],
                    num_idxs=T_B * P,
                    num_idxs_reg=T_B * P,
                    elem_size=D,
                    single_packet=False,
                    queue_num=(g + 2) % 4,
                )

                # stage-2 stationary operand: rows 0:64 = agg feats (from
                # psum1), rows 64:128 = x feats for this node block
                combo = spool.tile([2 * D, P], f32, tag="combo")
                nc.sync.dma_start(out=combo[D:2 * D, :], in_=xT[:, g * P:(g + 1) * P])

                psum1 = p1pool.tile([D, P], f32)
                for t in range(T):
                    oh = ohpool.tile([P, P], f32)
                    nc.vector.tensor_scalar(
                        out=oh[:],
                        in0=iota_f[:],
                        scalar1=dest_sb[:, g * T + t:g * T + t + 1],
                        scalar2=rve_sb[:, g * T + t:g * T + t + 1],
                        op0=mybir.AluOpType.is_equal,
                        op1=mybir.AluOpType.mult,
                    )
                    src = xga[:, t, :] if t < T_A else xgb[:, t - T_A, :]
                    nc.tensor.matmul(
                        psum1[:],
                        lhsT=src,
                        rhs=oh[:],
                        start=(t == 0),
                        stop=(t == T - 1),
                    )

                nc.scalar.copy(out=combo[0:D, :], in_=psum1[:])

                psum2 = p2pool.tile([P, D], f32, tag="psum2")
                nc.tensor.matmul(psum2[:], lhsT=combo[:], rhs=Wcat_sb[:], start=True, stop=True)

                out_sb = spool.tile([P, D], f32, tag="out_sb")
                if with_bias:
                    bias_sb = spool.tile([P, D], f32, tag="bias_sb")
                    nc.sync.dma_start(out=bias_sb[:], in_=bias[g * P:(g + 1) * P, :])
                    nc.vector.tensor_tensor(
                        out=out_sb[:], in0=psum2[:], in1=bias_sb[:],
                        op=mybir.AluOpType.add,
                    )
                else:
                    nc.scalar.copy(out=out_sb[:], in_=psum2[:])
                nc.sync.dma_start(out=out[g * P:(g + 1) * P, :], in_=out_sb[:])
    nc.compile()
    return nc


def _wrap_idx(slots):
    """[G, S] per-slot gather indices -> [G, 128, S//16] int16 wrapped.

    dma_gather reads index i from partition i%16, column i//16 (replicated
    across the 8 Q7 cores' 16-partition slices).
    """
    G, S = slots.shape
    w = slots.reshape(G, S // 16, 16).transpose(0, 2, 1)  # [G, 16, S//16]
    return np.tile(w, (1, 8, 1)).astype(np.int16)  # [G, 128, S//16]


def prepare(x, edge_index, W_msg, b_msg, W_self, b_self):
    x = np.asarray(x, dtype=np.float32)
    edge_index = np.asarray(edge_index)
    W_msg = np.asarray(W_msg, dtype=np.float32)
    W_self = np.asarray(W_self, dtype=np.float32)
    b_msg = np.asarray(b_msg, dtype=np.float32)
    b_self = np.asarray(b_self, dtype=np.float32)

    n = x.shape[0]
    GPC = _ceil_div(n, P * NC)
    G = NC * GPC
    NPAD = G * P
    NPC = GPC * P

    row = edge_index[0].astype(np.int64)
    col = edge_index[1].astype(np.int64)
    grp = (col // P).astype(np.int64)
    isB = row >= SPLIT

    cntA = np.bincount(grp[~isB], minlength=G)
    cntB = np.bincount(grp[isB], minlength=G)
    T_A = max(1, _ceil_div(int(cntA.max()), P))
    T_B = max(1, _ceil_div(int(cntB.max()), P))
    T = T_A + T_B

    deg = np.bincount(col, minlength=NPAD).astype(np.int64)
    rv_full = (1.0 / np.maximum(deg, 1)).astype(np.float32)

    # slot tables: [G, T_*128] gather index (0-padded), dest id (300-padded),
    # per-edge 1/deg (0-padded)
    slotsA = np.zeros((G, T_A * P), dtype=np.int64)
    destA = np.full((G, T_A * P), 300.0, dtype=np.float32)
    rveA = np.zeros((G, T_A * P), dtype=np.float32)
    slotsB = np.zeros((G, T_B * P), dtype=np.int64)
    destB = np.full((G, T_B * P), 300.0, dtype=np.float32)
    rveB = np.zeros((G, T_B * P), dtype=np.float32)

    for slots, destv, rvev, mask, base in (
        (slotsA, destA, rveA, ~isB, 0),
        (slotsB, destB, rveB, isB, SPLIT),
    ):
        r = row[mask]
        c = col[mask]
        g_of = grp[mask]
        o = np.argsort(g_of, kind="stable")
        r, c, g_of = r[o], c[o], g_of[o]
        cnt = np.bincount(g_of, minlength=G)
        starts = np.zeros(G + 1, dtype=np.int64)
        np.cumsum(cnt, out=starts[1:])
        pos = np.arange(len(r)) - starts[g_of]
        slots[g_of, pos] = r - base
        destv[g_of, pos] = (c - g_of * P).astype(np.float32)
        rvev[g_of, pos] = rv_full[c]

    idxA_w = _wrap_idx(slotsA)
    idxB_w = _wrap_idx(slotsB)

    # slot i of group -> partition i%128, tile i//128
    dest_pt = np.concatenate([destA, destB], axis=1).reshape(G, T, P).transpose(0, 2, 1)
    rve_pt = np.concatenate([rveA, rveB], axis=1).reshape(G, T, P).transpose(0, 2, 1)

    x_pad = np.zeros((NPAD, D), dtype=np.float32)
    x_pad[:n] = x
    x_lo = np.ascontiguousarray(x_pad[:SPLIT])
    x_hi = np.ascontiguousarray(x_pad[SPLIT:])
    n_lo, n_hi = x_lo.shape[0], x_hi.shape[0]

    Wcat = np.ascontiguousarray(np.concatenate([W_msg.T, W_self.T], axis=0))

    with_bias = bool(b_msg.any() or b_self.any())
    if with_bias:
        ind = (deg > 0).astype(np.float32)
        bias_full = b_self[None, :] + ind[:, None] * b_msg[None, :]

    in_maps = []
    for c in range(NC):
        gs = slice(c * GPC, (c + 1) * GPC)
        m = {
            "x_lo": x_lo,
            "x_hi": x_hi,
            "xT": np.ascontiguousarray(x_pad[c * NPC:(c + 1) * NPC].T),
            "idxA": np.ascontiguousarray(
                idxA_w[gs].transpose(1, 0, 2).reshape(P, GPC * T_A * 8)
            ),
            "idxB": np.ascontiguousarray(
                idxB_w[gs].transpose(1, 0, 2).reshape(P, GPC * T_B * 8)
            ),
            "dest": np.ascontiguousarray(
                dest_pt[gs].transpose(1, 0, 2).reshape(P, GPC * T)
            ),
            "rve": np.ascontiguousarray(
                rve_pt[gs].transpose(1, 0, 2).reshape(P, GPC * T)
            ),
            "Wcat": Wcat,
        }
        if with_bias:
            m["bias"] = np.ascontiguousarray(bias_full[c * NPC:(c + 1) * NPC])
        in_maps.append(m)

    meta = (T_A, T_B, GPC, n_lo, n_hi, with_bias, n)
    return meta, in_maps


def kernel(x, edge_index, W_msg, b_msg, W_self, b_self, _trace=False):
    from concourse.bass_utils import run_bass_kernel_spmd

    meta, in_maps = prepare(x, edge_index, W_msg, b_msg, W_self, b_self)
    T_A, T_B, GPC, n_lo, n_hi, with_bias, n = meta

    key = meta[:-1]
    if key not in _CACHE:
        _CACHE[key] = _build_bass(*key)
    nc = _CACHE[key]

    res = run_bass_kernel_spmd(nc, in_maps, list(range(NC)), trace=_trace)
    full = np.concatenate([res.results[c]["out"] for c in range(NC)], axis=0)
    out = np.ascontiguousarray(full[:n]).astype(np.float32, copy=False)
    if _trace:
        return out, res
    return out



# revision 2
# speedup vs baseline: 1.2754x; 1.2754x over previous
"""CustomSAGEConv on 8 Trainium2 NeuronCores — V3.

V3 = V2 (host-precomputed bf16 one-hots streamed from HBM, bf16-padded
x gather tables, 4-queue SWDGE rotation) plus:

  - per-SLOT tile counts: each core sorts its 49 destination groups by
    edge count (descending) and assigns them to program slots in that
    order; slot k's tile counts are the max over cores of each core's
    k-th largest group. Order statistics align across cores, so padding
    drops from the global max (931 tiles) to ~843 tiles/core.
  - gathers are merged pairwise (slots 2k, 2k+1 in one dma_gather per
    lo/hi table), halving SWDGE fixed overheads.
  - output rows are written in slot order and un-permuted on the host.
"""

import sys

for _p in ("/opt/trn_rl_repo", "/root/.axon_site/_ro/trn_rl_repo"):
    if _p not in sys.path:
        sys.path.insert(0, _p)

import numpy as np

P = 128
D = 64
NC = 8
SPLIT = 32768
EPAD = 128  # padded bf16 row elems (256B rows)

_CACHE = {}


def _ceil_div(a, b):
    return (a + b - 1) // b


def _build_bass(TA, TB, n_lo, n_hi, with_bias, repeat=1):
    import concourse.mybir as mybir
    import concourse.tile as tile
    from concourse import bacc

    TA = list(TA)
    TB = list(TB)
    GPC = len(TA)
    NPC = GPC * P
    sTA, sTB = sum(TA), sum(TB)
    # per-slot column offsets (tiles) into the flat idx/oh layouts
    oA = np.concatenate([[0], np.cumsum(TA)]).astype(int)
    oB = np.concatenate([[0], np.cumsum(TB)]).astype(int)
    oT = np.concatenate([[0], np.cumsum(np.array(TA) + np.array(TB))]).astype(int)
    pairs = [(2 * k, 2 * k + 1) if 2 * k + 1 < GPC else (2 * k,) for k in range(_ceil_div(GPC, 2))]
    maxA2 = max(sum(TA[s] for s in pr) for pr in pairs)
    maxB2 = max(sum(TB[s] for s in pr) for pr in pairs)

    nc = bacc.Bacc(num_swdge_queues=4)
    f32 = mybir.dt.float32
    bf16 = mybir.dt.bfloat16
    x_lo = nc.declare_dram_parameter("x_lo", [n_lo, EPAD], bf16, isOutput=False)
    x_hi = nc.declare_dram_parameter("x_hi", [n_hi, EPAD], bf16, isOutput=False)
    xT = nc.declare_dram_parameter("xT", [GPC * D, P], bf16, isOutput=False)
    idxA = nc.declare_dram_parameter("idxA", [P, sTA * 8], mybir.dt.int16, isOutput=False)
    idxB = nc.declare_dram_parameter("idxB", [P, sTB * 8], mybir.dt.int16, isOutput=False)
    oh = nc.declare_dram_parameter("oh", [P, (sTA + sTB) * P], bf16, isOutput=False)
    Wcat = nc.declare_dram_parameter("Wcat", [2 * D, D], bf16, isOutput=False)
    if with_bias:
        bias = nc.declare_dram_parameter("bias", [NPC, D], f32, isOutput=False)
    out = nc.declare_dram_parameter("out", [NPC, D], f32, isOutput=True)

    with tile.TileContext(nc) as tc:
        with (
            tc.tile_pool(name="const", bufs=1) as cpool,
            tc.tile_pool(name="gather", bufs=3) as gpool,
            tc.tile_pool(name="ohp", bufs=3) as ohpool,
            tc.tile_pool(name="small", bufs=4) as spool,
            tc.tile_pool(name="psum1", bufs=4, space="PSUM") as p1pool,
            tc.tile_pool(name="psum2", bufs=4, space="PSUM") as p2pool,
        ):
            idxA_sb = cpool.tile([P, sTA * 8], mybir.dt.int16)
            nc.sync.dma_start(out=idxA_sb[:], in_=idxA[:])
            idxB_sb = cpool.tile([P, sTB * 8], mybir.dt.int16)
            nc.sync.dma_start(out=idxB_sb[:], in_=idxB[:])
            Wcat_sb = cpool.tile([2 * D, D], bf16)
            nc.sync.dma_start(out=Wcat_sb[:], in_=Wcat[:])

            for rep in range(repeat):
                for pi, pr in enumerate(pairs):
                    nA = sum(TA[s] for s in pr)
                    nB = sum(TB[s] for s in pr)
                    xga = gpool.tile([P, maxA2, EPAD], bf16, tag="xga")
                    nc.gpsimd.dma_gather(
                        out_ap=xga[:, 0:nA, :],
                        in_ap=x_lo[:],
                        idxs_ap=idxA_sb[:, oA[pr[0]] * 8:(oA[pr[0]] + nA) * 8],
                        num_idxs=nA * P,
                        num_idxs_reg=nA * P,
                        elem_size=EPAD,
                        single_packet=False,
                        queue_num=(2 * pi) % 4,
                    )
                    xgb = gpool.tile([P, maxB2, EPAD], bf16, tag="xgb")
                    nc.gpsimd.dma_gather(
                        out_ap=xgb[:, 0:nB, :],
                        in_ap=x_hi[:],
                        idxs_ap=idxB_sb[:, oB[pr[0]] * 8:(oB[pr[0]] + nB) * 8],
                        num_idxs=nB * P,
                        num_idxs_reg=nB * P,
                        elem_size=EPAD,
                        single_packet=False,
                        queue_num=(2 * pi + 1) % 4,
                    )

                    for si, g in enumerate(pr):
                        T_a, T_b = TA[g], TB[g]
                        aoff = oA[g] - oA[pr[0]]
                        boff = oB[g] - oB[pr[0]]

                        oh_sb = ohpool.tile([P, (T_a + T_b) * P], bf16, tag=f"oh{si}")
                        nc.sync.dma_start(out=oh_sb[:], in_=oh[:, oT[g] * P:oT[g + 1] * P])

                        combo = spool.tile([2 * D, P], bf16, tag=f"combo{si}")
                        nc.sync.dma_start(out=combo[D:2 * D, :], in_=xT[g * D:(g + 1) * D, :])

                        psum1 = p1pool.tile([D, P], f32)
                        for t in range(T_a + T_b):
                            if t < T_a:
                                src = xga[:, aoff + t, 0:D]
                            else:
                                src = xgb[:, boff + t - T_a, 0:D]
                            nc.tensor.matmul(
                                psum1[:],
                                lhsT=src,
                                rhs=oh_sb[:, t * P:(t + 1) * P],
                                start=(t == 0),
                                stop=(t == T_a + T_b - 1),
                            )

                        nc.scalar.copy(out=combo[0:D, :], in_=psum1[:])

                        psum2 = p2pool.tile([P, D], f32, tag="psum2")
                        nc.tensor.matmul(psum2[:], lhsT=combo[:], rhs=Wcat_sb[:], start=True, stop=True)

                        out_sb = spool.tile([P, D], f32, tag=f"out_sb{si}")
                        if with_bias:
                            bias_sb = spool.tile([P, D], f32, tag=f"bias_sb{si}")
                            nc.sync.dma_start(out=bias_sb[:], in_=bias[g * P:(g + 1) * P, :])
                            nc.vector.tensor_tensor(
                                out=out_sb[:], in0=psum2[:], in1=bias_sb[:],
                                op=mybir.AluOpType.add,
                            )
                        else:
                            nc.scalar.copy(out=out_sb[:], in_=psum2[:])
                        nc.sync.dma_start(out=out[g * P:(g + 1) * P, :], in_=out_sb[:])
    nc.compile()
    return nc


def _wrap_idx_flat(slots):
    """[S] slot gather indices -> [128, S//16] int16 wrapped (x8 cores)."""
    S = len(slots)
    w = slots.reshape(S // 16, 16).T  # [16, S//16]
    return np.tile(w, (8, 1)).astype(np.int16)


def prepare(x, edge_index, W_msg, b_msg, W_self, b_self):
    import ml_dtypes

    x = np.asarray(x, dtype=np.float32)
    edge_index = np.asarray(edge_index)
    W_msg = np.asarray(W_msg, dtype=np.float32)
    W_self = np.asarray(W_self, dtype=np.float32)
    b_msg = np.asarray(b_msg, dtype=np.float32)
    b_self = np.asarray(b_self, dtype=np.float32)

    n = x.shape[0]
    GPC = _ceil_div(n, P * NC)
    G = NC * GPC
    NPAD = G * P

    row = edge_index[0].astype(np.int64)
    col = edge_index[1].astype(np.int64)
    grp = (col // P).astype(np.int64)
    isB = row >= SPLIT

    cntA = np.bincount(grp[~isB], minlength=G).reshape(NC, GPC)
    cntB = np.bincount(grp[isB], minlength=G).reshape(NC, GPC)

    # slot assignment: each core's groups sorted by total count desc
    order = np.argsort(-(cntA + cntB), axis=1, kind="stable")  # [NC, GPC] slot->group
    cA_s = np.take_along_axis(cntA, order, 1)
    cB_s = np.take_along_axis(cntB, order, 1)
    TA = np.maximum(1, np.ceil(cA_s.max(axis=0) / P).astype(int))  # [GPC]
    TB = np.maximum(1, np.ceil(cB_s.max(axis=0) / P).astype(int))
    sTA, sTB = int(TA.sum()), int(TB.sum())
    oA = np.concatenate([[0], np.cumsum(TA)]).astype(int)
    oB = np.concatenate([[0], np.cumsum(TB)]).astype(int)
    oT = np.concatenate([[0], np.cumsum(TA + TB)]).astype(int)

    deg = np.bincount(col, minlength=NPAD).astype(np.int64)
    rv_full = (1.0 / np.maximum(deg, 1)).astype(np.float32)

    # global slot id of (core, group): inverse of order
    slot_of = np.empty_like(order)
    np.put_along_axis(slot_of, order, np.arange(GPC)[None, :].repeat(NC, 0), 1)

    slotsA = np.zeros((NC, sTA * P), dtype=np.int64)
    slotsB = np.zeros((NC, sTB * P), dtype=np.int64)
    ohw = np.zeros((NC, (sTA + sTB) * P, P), dtype=np.float32)

    core_of_edge = grp // GPC
    gl = grp % GPC  # group local id within core

    for slots, mask, base, sideB in (
        (slotsA, ~isB, 0, False),
        (slotsB, isB, SPLIT, True),
    ):
        r = row[mask]
        c = col[mask]
        cr = core_of_edge[mask]
        sl = slot_of[cr, gl[mask]]
        # sort by (core, slot, src)
        o = np.lexsort((r, sl, cr))
        r, c, cr, sl = r[o], c[o], cr[o], sl[o]
        key = cr * GPC + sl
        cnt = np.bincount(key, minlength=NC * GPC)
        starts = np.zeros(NC * GPC + 1, dtype=np.int64)
        np.cumsum(cnt, out=starts[1:])
        pos = np.arange(len(r)) - starts[key]
        if sideB:
            slotbase = (oB[sl] * P + pos)
            ohslot = (oT[sl] + TA[sl]) * P + pos
        else:
            slotbase = (oA[sl] * P + pos)
            ohslot = oT[sl] * P + pos
        slots[cr, slotbase] = r - base
        ohw[cr, ohslot, c % P] = rv_full[c]

    # slot s (within its flat region) -> partition s%128, tile s//128
    # one-hot layout: [P(slot partition), (sTA+sTB)*P] with tile t at
    # cols t*128:(t+1)*128 (flat tile index across all slots)
    x_pad = np.zeros((NPAD, EPAD), dtype=np.float32)
    x_pad[:n, :D] = x
    x_pad_bf = x_pad.astype(ml_dtypes.bfloat16)
    x_lo = np.ascontiguousarray(x_pad_bf[:SPLIT])
    x_hi = np.ascontiguousarray(x_pad_bf[SPLIT:])
    n_lo, n_hi = x_lo.shape[0], x_hi.shape[0]

    Wcat = np.ascontiguousarray(
        np.concatenate([W_msg.T, W_self.T], axis=0)
    ).astype(ml_dtypes.bfloat16)

    with_bias = bool(b_msg.any() or b_self.any())
    if with_bias:
        ind = (deg > 0).astype(np.float32)
        bias_full = (b_self[None, :] + ind[:, None] * b_msg[None, :]).reshape(NC, GPC, P, D)

    in_maps = []
    TT = sTA + sTB
    for cc in range(NC):
        ohc = ohw[cc].reshape(TT, P, P).transpose(1, 0, 2).reshape(P, TT * P)
        # xT in slot order: slot k holds group order[cc, k]
        xTc = x_pad[cc * GPC * P:(cc + 1) * GPC * P, :D].reshape(GPC, P, D)
        xTc = xTc[order[cc]].transpose(0, 2, 1).reshape(GPC * D, P)
        m = {
            "x_lo": x_lo,
            "x_hi": x_hi,
            "xT": np.ascontiguousarray(xTc).astype(ml_dtypes.bfloat16),
            "idxA": _wrap_idx_flat(slotsA[cc]),
            "idxB": _wrap_idx_flat(slotsB[cc]),
            "oh": np.ascontiguousarray(ohc).astype(ml_dtypes.bfloat16),
            "Wcat": Wcat,
        }
        if with_bias:
            m["bias"] = np.ascontiguousarray(
                bias_full[cc][order[cc]].reshape(GPC * P, D)
            )
        in_maps.append(m)

    meta = (tuple(TA), tuple(TB), n_lo, n_hi, with_bias)
    return meta, in_maps, order, n, GPC


def kernel(x, edge_index, W_msg, b_msg, W_self, b_self, _trace=False, _repeat=1):
    from concourse.bass_utils import run_bass_kernel_spmd

    meta, in_maps, order, n, GPC = prepare(x, edge_index, W_msg, b_msg, W_self, b_self)

    key = meta + (_repeat,)
    if key not in _CACHE:
        _CACHE[key] = _build_bass(*meta, repeat=_repeat)
    nc = _CACHE[key]

    res = run_bass_kernel_spmd(nc, in_maps, list(range(NC)), trace=_trace)
    full = np.empty((NC * GPC * P, D), dtype=np.float32)
    for cc in range(NC):
        o = res.results[cc]["out"].reshape(GPC, P, D)
        blk = full[cc * GPC * P:(cc + 1) * GPC * P].reshape(GPC, P, D)
        blk[order[cc]] = o  # slot k holds group order[cc, k]
    out = np.ascontiguousarray(full[:n]).astype(np.float32, copy=False)
    if _trace:
        return out, res
    return out
